# revision 11
# baseline (speedup 1.0000x reference)
"""Trainium2 Bass kernel for windowed (block-diagonal) multi-head video attention.

Problem: x:[2,8192,1024] -> qkv proj -> 3D-window (2,8,8) attention over a
(8,32,32) token grid, 16 heads x 64 dim -> out proj -> [2,8192,1024].

Sharding: 8 cores, data-parallel over (batch, t-window-group).  Token order is
(t,h,w)-major, so the slab x[b, it*2048:(it+1)*2048, :] is contiguous and holds
exactly the 16 independent (h,w)-windows with t in {2it, 2it+1}.  Each core:
  - DMA-gathers each window's 128 tokens as a [128,1024] tile (strided AP)
  - PE-transposes x_win -> x^T (contraction dim on partitions)
  - QKV projection: Q,K produced head-transposed [oc,tok]; V token-major with a
    per-head ones column appended (65-stride layout)
  - S^T = K_h Q_h^T per head (K=64), exp on ACT, A·V matmul where the ones row
    yields the softmax denominator for free; normalize with reciprocal +
    gpsimd partition-broadcast + DVE multiply
  - out projection, DMA-scatter back to token order
Weights are pre-transposed on the host; biases (zero in this problem) are
supported via rank-1 (K=1) accumulation matmuls, compiled only when nonzero.
"""

import sys

for _p in ("/opt/trn_rl_repo",):
    if _p not in sys.path:
        sys.path.insert(0, _p)

import numpy as np

B, T, H, W = 2, 8, 32, 32
C, NH, HD = 1024, 16, 64
WT, WH, WW = 2, 8, 8
N = T * H * W              # 8192 tokens
SCALE = HD ** -0.5
NCORES = 8
SLAB = N // (T // WT)      # 2048 tokens per (b, it) slab
NWIN = (H // WH) * (W // WW)   # 16 windows per slab
M = WT * WH * WW           # 128 tokens per window
KC = C // 128              # 8 contraction chunks

_BUILD_CACHE = {}


def _split_drain_waits(nc, mybir, cap=1, event_cap=2):
    """This walrus build accepts only one sem wait per TPB instruction
    (Tile's scheduler attaches up to 3).  Move the excess onto
    InstEventSemaphore carriers (which hold 2) inserted right before the
    over-subscribed instruction on the same engine — the engine blocks on the
    carriers first, so semantics are unchanged."""
    for f in nc.m.functions:
        for bb in f.blocks:
            i = 0
            while i < len(bb.instructions):
                ins = bb.instructions[i]
                si = ins.sync_info
                my_cap = (
                    event_cap
                    if type(ins).__name__ == "InstEventSemaphore"
                    else cap
                )
                if si is not None and si.on_wait and len(si.on_wait) > my_cap:
                    waits = list(si.on_wait)
                    si.on_wait = waits[:my_cap]
                    extra = waits[my_cap:]
                    carriers = []
                    while extra:
                        chunk, extra = extra[:event_cap], extra[event_cap:]
                        ev = mybir.InstEventSemaphore(
                            name=f"I-{nc.next_id()}-waitsplit", ins=[], outs=[]
                        )
                        ev.engine = ins.engine
                        ev.sync_info = mybir.SyncInfo(
                            on_wait=list(chunk), on_update=[]
                        )
                        nc.register_instruction(ev)
                        carriers.append(ev)
                    bb.instructions[i:i] = carriers
                    i += len(carriers)
                i += 1


def _build(has_qkvb, has_projb):
    import concourse.bass as bass
    import concourse.tile as tile
    from concourse import mybir
    from concourse.masks import make_identity

    f32 = mybir.dt.float32

    nc = bass.Bass("TRN2", target_bir_lowering=False, debug=False)
    xs = nc.dram_tensor("xs", [SLAB, C], f32, kind="ExternalInput")
    wqkvT = nc.dram_tensor("wqkvT", [C, 3 * C], f32, kind="ExternalInput")
    projT = nc.dram_tensor("projT", [C, C], f32, kind="ExternalInput")
    if has_qkvb:
        qkvb = nc.dram_tensor("qkvb", [1, 3 * C], f32, kind="ExternalInput")
    if has_projb:
        projb = nc.dram_tensor("projb", [1, C], f32, kind="ExternalInput")
    out = nc.dram_tensor("out", [SLAB, C], f32, kind="ExternalOutput")

    # window gather/scatter views: slab token idx = tt*1024 + hh*32 + ww in a
    # [2, (4,8), (4,8)] = (tt, ih hh, iw ww) decomposition; window = (ih, iw)
    xs_v = xs.ap().rearrange(
        "(tt ih hh iw ww) c -> ih iw tt hh ww c", tt=WT, ih=4, hh=WH, iw=4, ww=WW
    )
    out_v = out.ap().rearrange(
        "(tt ih hh iw ww) c -> ih iw tt hh ww c", tt=WT, ih=4, hh=WH, iw=4, ww=WW
    )

    with tile.TileContext(nc) as tc:
        with (
            tc.tile_pool(name="wq", bufs=1) as wq_pool,
            tc.tile_pool(name="wp", bufs=1) as wp_pool,
            tc.tile_pool(name="const", bufs=1) as const_pool,
            tc.tile_pool(name="xw", bufs=2) as xw_pool,
            tc.tile_pool(name="xT", bufs=2) as xT_pool,
            tc.tile_pool(name="qk", bufs=1) as qk_pool,
            tc.tile_pool(name="v65", bufs=1) as v_pool,
            tc.tile_pool(name="E", bufs=1) as e_pool,
            tc.tile_pool(name="rR", bufs=4) as r_pool,
            tc.tile_pool(name="owT", bufs=2) as ow_pool,
            tc.tile_pool(name="o", bufs=2) as o_pool,
            tc.tile_pool(name="psA", bufs=4, space="PSUM") as psA,
            tc.tile_pool(name="psB", bufs=4, space="PSUM") as psB,
        ):
            ident = const_pool.tile([128, 128], f32)
            make_identity(nc, ident[:])
            ones64 = const_pool.tile([1, 64], f32)
            nc.vector.memset(ones64[:], 1.0)

            wq_sb = wq_pool.tile([128, KC, 3 * C], f32)
            nc.sync.dma_start(
                wq_sb[:], wqkvT.ap().rearrange("(k p) o -> p k o", p=128)
            )
            wp_sb = wp_pool.tile([128, KC, C], f32)
            nc.sync.dma_start(
                wp_sb[:], projT.ap().rearrange("(k p) o -> p k o", p=128)
            )
            if has_qkvb or has_projb:
                ones = const_pool.tile([1, 128], f32)
                nc.vector.memset(ones[:], 1.0)
            if has_qkvb:
                qkvb_sb = const_pool.tile([1, 3 * C], f32)
                nc.sync.dma_start(qkvb_sb[:], qkvb.ap())
            if has_projb:
                projb_sb = const_pool.tile([1, C], f32)
                nc.sync.dma_start(projb_sb[:], projb.ap())

            for ih in range(4):
                for iw in range(4):
                    # 1) gather window tokens [128, 1024] (2 DMAs: AP balancer
                    #    handles at most 3 dims, so one per tt slice)
                    xw = xw_pool.tile([128, C], f32)
                    for tt in range(WT):
                        nc.sync.dma_start(
                            xw[64 * tt : 64 * (tt + 1), :], xs_v[ih, iw, tt]
                        )

                    # 2) transpose x_win -> xT [c (8 chunks of 128), tok]
                    xT = xT_pool.tile([128, KC, 128], f32)
                    for tb in range(2):
                        ps = psA.tile([128, 512], f32, tag="psA")
                        for j in range(4):
                            jj = 4 * tb + j
                            nc.tensor.transpose(
                                ps[:, 128 * j : 128 * (j + 1)],
                                xw[:, 128 * jj : 128 * (jj + 1)],
                                ident[:],
                            )
                        nc.scalar.copy(xT[:, 4 * tb : 4 * tb + 4, :], ps[:])

                    # 3) Q,K head-transposed.  psum bank = [oc 128, tok 128] x4
                    #    chunks; oc chunk c holds heads (2c, 2c+1).  Evict into
                    #    a 64-partition per-head layout (slot 2c / 2c+1 for
                    #    rows 0:64 / 64:128) so the S matmuls never use
                    #    partition-base-64 operands — mixing base-0/base-64
                    #    matmul operands (auto tile_position) hangs trn2.
                    qkT = qk_pool.tile([64, 4 * KC, 128], f32)
                    qkTv = qkT[:].rearrange("p (s two) t -> p s two t", two=2)
                    for bank in range(4):
                        ps = psA.tile([128, 512], f32, tag="psA")
                        for sub in range(4):
                            oc = 4 * bank + sub
                            for k in range(KC):
                                nc.tensor.matmul(
                                    ps[:, 128 * sub : 128 * (sub + 1)],
                                    wq_sb[:, k, 128 * oc : 128 * (oc + 1)],
                                    xT[:, k, :],
                                    start=(k == 0),
                                    stop=(k == KC - 1 and not has_qkvb),
                                )
                            if has_qkvb:
                                nc.tensor.matmul(
                                    ps[:, 128 * sub : 128 * (sub + 1)],
                                    qkvb_sb[0:1, 128 * oc : 128 * (oc + 1)],
                                    ones[0:1, 0:128],
                                    start=False,
                                    stop=True,
                                )
                        # fold the attention scale into Q's eviction (DVE:
                        # partition-shifted strided copies)
                        sc = SCALE if bank < 2 else 1.0
                        psv = ps[:].rearrange("p (c t) -> p c t", t=128)
                        nc.vector.tensor_scalar_mul(
                            qkTv[:, 4 * bank : 4 * bank + 4, 0, :],
                            psv[0:64, :, :],
                            sc,
                        )
                        nc.vector.tensor_scalar_mul(
                            qkTv[:, 4 * bank : 4 * bank + 4, 1, :],
                            psv[64:128, :, :],
                            sc,
                        )

                    # 4) V token-major with ones column per head (stride 65)
                    v65 = v_pool.tile([128, NH, HD + 1], f32)
                    nc.vector.memset(v65[:, :, HD : HD + 1], 1.0)
                    for nk in range(2):
                        ps = psA.tile([128, 512], f32, tag="psA")
                        for k in range(KC):
                            nc.tensor.matmul(
                                ps[:],
                                xT[:, k, :],
                                wq_sb[:, k, 2 * C + 512 * nk : 2 * C + 512 * (nk + 1)],
                                start=(k == 0),
                                stop=(k == KC - 1 and not has_qkvb),
                            )
                        if has_qkvb:
                            nc.tensor.matmul(
                                ps[:],
                                ones[0:1, 0:128],
                                qkvb_sb[0:1, 2 * C + 512 * nk : 2 * C + 512 * (nk + 1)],
                                start=False,
                                stop=True,
                            )
                        for h8 in range(8):
                            h = 8 * nk + h8
                            nc.scalar.copy(
                                v65[:, h, 0:HD], ps[:, 64 * h8 : 64 * (h8 + 1)]
                            )

                    # 5) attention per 4-head bank
                    E = e_pool.tile([128, NH, 128], f32)
                    owT = ow_pool.tile([128, KC, 128], f32)
                    for hb in range(4):
                        psS = psB.tile([128, 512], f32, tag="psB")
                        for m in range(4):
                            h = 4 * hb + m
                            # S^T[kt,qt] = (K_h^T).T @ Q_h^T, K=64 (base 0)
                            nc.tensor.matmul(
                                psS[:, 128 * m : 128 * (m + 1)],
                                qkT[:, NH + h, :],
                                qkT[:, h, :],
                                start=True,
                                stop=True,
                            )
                        nc.scalar.activation(
                            E[:, 4 * hb : 4 * hb + 4, :],
                            psS[:],
                            mybir.ActivationFunctionType.Exp,
                        )
                        psV = psB.tile([128, 512], f32, tag="psB")
                        for m in range(4):
                            h = 4 * hb + m
                            # rows 0..63 = V^T E (unnormalized ow^T), row 64 = denom
                            nc.tensor.matmul(
                                psV[0:65, 128 * m : 128 * (m + 1)],
                                v65[:, h, :],
                                E[:, h, :],
                                start=True,
                                stop=True,
                            )
                        r = r_pool.tile([1, 512], f32, tag="r")
                        nc.vector.reciprocal(r[:], psV[64:65, :])
                        # broadcast r along partitions: K=1 matmul ones64^T @ r
                        Rp = psB.tile([64, 512], f32, tag="psB")
                        nc.tensor.matmul(
                            Rp[:], ones64[:], r[:], start=True, stop=True
                        )
                        R = r_pool.tile([64, 512], f32, tag="R")
                        nc.scalar.copy(R[:], Rp[:])
                        for m in range(4):
                            h = 4 * hb + m
                            po = (h % 2) * 64
                            nc.vector.tensor_tensor(
                                owT[po : po + 64, h // 2, :],
                                psV[0:64, 128 * m : 128 * (m + 1)],
                                R[:, 128 * m : 128 * (m + 1)],
                                op=mybir.AluOpType.mult,
                            )

                    # 6) out projection [tok, oc2]
                    otile = o_pool.tile([128, C], f32)
                    for nk in range(2):
                        ps = psA.tile([128, 512], f32, tag="psA")
                        for k in range(KC):
                            nc.tensor.matmul(
                                ps[:],
                                owT[:, k, :],
                                wp_sb[:, k, 512 * nk : 512 * (nk + 1)],
                                start=(k == 0),
                                stop=(k == KC - 1 and not has_projb),
                            )
                        if has_projb:
                            nc.tensor.matmul(
                                ps[:],
                                ones[0:1, 0:128],
                                projb_sb[0:1, 512 * nk : 512 * (nk + 1)],
                                start=False,
                                stop=True,
                            )
                        nc.scalar.copy(otile[:, 512 * nk : 512 * (nk + 1)], ps[:])

                    # 7) scatter back to token order
                    for tt in range(WT):
                        nc.sync.dma_start(
                            out_v[ih, iw, tt], otile[64 * tt : 64 * (tt + 1), :]
                        )

    _split_drain_waits(nc, mybir)
    return nc


def _get_nc(has_qkvb, has_projb):
    key = (has_qkvb, has_projb)
    if key not in _BUILD_CACHE:
        _BUILD_CACHE[key] = _build(has_qkvb, has_projb)
    return _BUILD_CACHE[key]


def kernel(x, qkv_w, qkv_b, proj_w, proj_b, t, h, w, **_unused):
    from concourse.bass_utils import run_bass_kernel_spmd

    x = np.asarray(x, dtype=np.float32)
    qkv_w = np.asarray(qkv_w, dtype=np.float32)
    qkv_b = np.asarray(qkv_b, dtype=np.float32)
    proj_w = np.asarray(proj_w, dtype=np.float32)
    proj_b = np.asarray(proj_b, dtype=np.float32)
    assert x.shape == (B, N, C), x.shape
    assert int(t) == T and int(h) == H and int(w) == W

    has_qkvb = bool(np.any(qkv_b))
    has_projb = bool(np.any(proj_b))
    nc = _get_nc(has_qkvb, has_projb)

    wqkvT = np.ascontiguousarray(qkv_w.T)
    projT = np.ascontiguousarray(proj_w.T)

    in_maps = []
    for core in range(NCORES):
        b, it = divmod(core, T // WT)
        im = {
            "xs": np.ascontiguousarray(x[b, it * SLAB : (it + 1) * SLAB, :]),
            "wqkvT": wqkvT,
            "projT": projT,
        }
        if has_qkvb:
            im["qkvb"] = qkv_b.reshape(1, 3 * C)
        if has_projb:
            im["projb"] = proj_b.reshape(1, C)
        in_maps.append(im)

    res = run_bass_kernel_spmd(nc, in_maps, core_ids=list(range(NCORES)))

    y = np.empty((B, N, C), dtype=np.float32)
    for core in range(NCORES):
        b, it = divmod(core, T // WT)
        y[b, it * SLAB : (it + 1) * SLAB, :] = res.results[core]["out"]
    return y


# revision 18
# speedup vs baseline: 1.7583x; 1.7583x over previous
"""Trainium2 Bass kernel for windowed (block-diagonal) multi-head video attention.

Problem: x:[2,8192,1024] -> qkv proj -> 3D-window (2,8,8) attention over a
(8,32,32) token grid, 16 heads x 64 dim -> out proj -> [2,8192,1024].

Sharding: 8 cores, data-parallel over (batch, t-window-group).  Token order is
(t,h,w)-major, so the slab x[b, it*2048:(it+1)*2048, :] is contiguous and holds
exactly the 16 independent (h,w)-windows with t in {2it, 2it+1}.  Each core:
  - DMA-gathers each window's 128 tokens as a [128,1024] tile (strided AP)
  - PE-transposes x_win -> x^T (contraction dim on partitions)
  - QKV projection: Q,K produced head-transposed [oc,tok]; V token-major with a
    per-head ones column appended (65-stride layout)
  - S^T = K_h Q_h^T per head (K=64), exp on ACT, A·V matmul where the ones row
    yields the softmax denominator for free; normalize with reciprocal +
    gpsimd partition-broadcast + DVE multiply
  - out projection, DMA-scatter back to token order
Weights are pre-transposed on the host; biases (zero in this problem) are
supported via rank-1 (K=1) accumulation matmuls, compiled only when nonzero.
"""

import sys

for _p in ("/opt/trn_rl_repo",):
    if _p not in sys.path:
        sys.path.insert(0, _p)

import numpy as np

B, T, H, W = 2, 8, 32, 32
C, NH, HD = 1024, 16, 64
WT, WH, WW = 2, 8, 8
N = T * H * W              # 8192 tokens
SCALE = HD ** -0.5
NCORES = 8
SLAB = N // (T // WT)      # 2048 tokens per (b, it) slab
NWIN = (H // WH) * (W // WW)   # 16 windows per slab
M = WT * WH * WW           # 128 tokens per window
KC = C // 128              # 8 contraction chunks

_BUILD_CACHE = {}


def _split_drain_waits(nc, mybir, cap=1, event_cap=2):
    """This walrus build accepts only one sem wait per TPB instruction
    (Tile's scheduler attaches up to 3).  Move the excess onto
    InstEventSemaphore carriers (which hold 2) inserted right before the
    over-subscribed instruction on the same engine — the engine blocks on the
    carriers first, so semantics are unchanged."""
    for f in nc.m.functions:
        for bb in f.blocks:
            i = 0
            while i < len(bb.instructions):
                ins = bb.instructions[i]
                si = ins.sync_info
                my_cap = (
                    event_cap
                    if type(ins).__name__ == "InstEventSemaphore"
                    else cap
                )
                if si is not None and si.on_wait and len(si.on_wait) > my_cap:
                    waits = list(si.on_wait)
                    si.on_wait = waits[:my_cap]
                    extra = waits[my_cap:]
                    carriers = []
                    while extra:
                        chunk, extra = extra[:event_cap], extra[event_cap:]
                        ev = mybir.InstEventSemaphore(
                            name=f"I-{nc.next_id()}-waitsplit", ins=[], outs=[]
                        )
                        ev.engine = ins.engine
                        ev.sync_info = mybir.SyncInfo(
                            on_wait=list(chunk), on_update=[]
                        )
                        nc.register_instruction(ev)
                        carriers.append(ev)
                    bb.instructions[i:i] = carriers
                    i += len(carriers)
                i += 1


def _build(has_qkvb, has_projb, use_f32r=True):
    import concourse.bass as bass
    import concourse.tile as tile
    from concourse import mybir
    from concourse.masks import make_identity

    f32 = mybir.dt.float32
    fpr = mybir.dt.float32r if use_f32r else f32

    nc = bass.Bass("TRN2", target_bir_lowering=False, debug=False)
    xs = nc.dram_tensor("xs", [SLAB, C], f32, kind="ExternalInput")
    # weight dtype f32r: same 4-byte layout, PE rounds on read (tf32-like)
    wqkvT = nc.dram_tensor("wqkvT", [C, 3 * C], fpr, kind="ExternalInput")
    projT = nc.dram_tensor("projT", [C, C], fpr, kind="ExternalInput")
    if has_qkvb:
        qkvb = nc.dram_tensor("qkvb", [1, 3 * C], fpr, kind="ExternalInput")
    if has_projb:
        projb = nc.dram_tensor("projb", [1, C], fpr, kind="ExternalInput")
    out = nc.dram_tensor("out", [SLAB, C], f32, kind="ExternalOutput")

    # window gather/scatter views: slab token idx = tt*1024 + hh*32 + ww in a
    # [2, (4,8), (4,8)] = (tt, ih hh, iw ww) decomposition; window = (ih, iw)
    xs_v = xs.ap().rearrange(
        "(tt ih hh iw ww) c -> ih iw tt hh ww c", tt=WT, ih=4, hh=WH, iw=4, ww=WW
    )
    out_v = out.ap().rearrange(
        "(tt ih hh iw ww) c -> ih iw tt hh ww c", tt=WT, ih=4, hh=WH, iw=4, ww=WW
    )

    # windows processed in pairs: tok dim = 256 so the f32r matmuls hit the
    # 1 cyc/row regime (ap_size >= 256); attention blocks stay per-window
    GW = 2
    TOKG = 128 * GW

    with tile.TileContext(nc) as tc:
        with (
            tc.tile_pool(name="wq", bufs=1) as wq_pool,
            tc.tile_pool(name="wp", bufs=1) as wp_pool,
            tc.tile_pool(name="const", bufs=1) as const_pool,
            tc.tile_pool(name="xw", bufs=1) as xw_pool,
            tc.tile_pool(name="xT", bufs=1) as xT_pool,
            tc.tile_pool(name="qk", bufs=1) as qk_pool,
            tc.tile_pool(name="v65", bufs=1) as v_pool,
            tc.tile_pool(name="E", bufs=2) as e_pool,
            tc.tile_pool(name="rR", bufs=2) as r_pool,
            tc.tile_pool(name="owT", bufs=1) as ow_pool,
            tc.tile_pool(name="o", bufs=1) as o_pool,
            tc.tile_pool(name="psA", bufs=4, space="PSUM") as psA,
            tc.tile_pool(name="psB", bufs=4, space="PSUM") as psB,
        ):
            ident = const_pool.tile([128, 128], f32)
            make_identity(nc, ident[:])
            ones64f = const_pool.tile([1, 64], f32)
            nc.vector.memset(ones64f[:], 1.0)
            ones64 = const_pool.tile([1, 64], fpr)
            nc.scalar.copy(ones64[:], ones64f[:])

            wq_sb = wq_pool.tile([128, KC, 3 * C], fpr)
            nc.sync.dma_start(
                wq_sb[:], wqkvT.ap().rearrange("(k p) o -> p k o", p=128)
            )
            wp_sb = wp_pool.tile([128, KC, C], fpr)
            nc.sync.dma_start(
                wp_sb[:], projT.ap().rearrange("(k p) o -> p k o", p=128)
            )
            if has_qkvb or has_projb:
                onesf = const_pool.tile([1, 128], f32)
                nc.vector.memset(onesf[:], 1.0)
                ones = const_pool.tile([1, 128], fpr)
                nc.scalar.copy(ones[:], onesf[:])
            if has_qkvb:
                qkvb_sb = const_pool.tile([1, 3 * C], fpr)
                nc.sync.dma_start(qkvb_sb[:], qkvb.ap())
            if has_projb:
                projb_sb = const_pool.tile([1, C], fpr)
                nc.sync.dma_start(projb_sb[:], projb.ap())

            for grp in range(NWIN // GW):
                wins = [(divmod(GW * grp + w, 4)) for w in range(GW)]

                # 1+2) per window: gather tokens, PE-transpose into the
                # group x^T tile [c-chunk partitions, (chunk, tok)] (f32r)
                xT = xT_pool.tile([128, KC, TOKG], fpr)
                for w, (ih, iw) in enumerate(wins):
                    xw = xw_pool.tile([128, C], f32)
                    for tt in range(WT):
                        nc.sync.dma_start(
                            xw[64 * tt : 64 * (tt + 1), :], xs_v[ih, iw, tt]
                        )
                    for tb in range(2):
                        ps = psA.tile([128, 512], f32, tag="psA")
                        for j in range(4):
                            jj = 4 * tb + j
                            nc.tensor.transpose(
                                ps[:, 128 * j : 128 * (j + 1)],
                                xw[:, 128 * jj : 128 * (jj + 1)],
                                ident[:],
                            )
                        psv = ps[:].rearrange("p (c t) -> p c t", t=128)
                        nc.scalar.copy(
                            xT[:].rearrange("p k (g t) -> p k g t", g=GW)[
                                :, 4 * tb : 4 * tb + 4, w, :
                            ],
                            psv[:],
                        )

                # 3) Q,K head-transposed: psum bank [oc 128, tok 256] x2 chunks.
                # Evict to 64-partition per-head layout (slot 2c+parity) so S
                # matmuls never use partition-base-64 operands (mixing base-0
                # and base-64 matmul operands hangs trn2).  qkT stays plain f32
                # (S matmuls are exact f32; f32r has no gain at N=128).
                qkT = qk_pool.tile([64, 4 * KC, TOKG], f32)
                qkTv = qkT[:].rearrange("p (s two) t -> p s two t", two=2)
                for bank in range(8):
                    ps = psA.tile([128, 512], f32, tag="psA")
                    for sub in range(2):
                        oc = 2 * bank + sub
                        for k in range(KC):
                            nc.tensor.matmul(
                                ps[:, TOKG * sub : TOKG * (sub + 1)],
                                wq_sb[:, k, 128 * oc : 128 * (oc + 1)],
                                xT[:, k, :],
                                start=(k == 0),
                                stop=(k == KC - 1 and not has_qkvb),
                            )
                        if has_qkvb:
                            nc.tensor.matmul(
                                ps[:, TOKG * sub : TOKG * (sub + 1)],
                                qkvb_sb[0:1, 128 * oc : 128 * (oc + 1)],
                                ones[0:1, 0:TOKG],
                                start=False,
                                stop=True,
                            )
                    sc = SCALE if bank < 4 else 1.0
                    psv = ps[:].rearrange("p (c t) -> p c t", t=TOKG)
                    nc.vector.tensor_scalar_mul(
                        qkTv[:, 2 * bank : 2 * bank + 2, 0, :], psv[0:64, :, :], sc
                    )
                    nc.vector.tensor_scalar_mul(
                        qkTv[:, 2 * bank : 2 * bank + 2, 1, :], psv[64:128, :, :], sc
                    )

                # 4) V token-major per window, ones column per head (stride 65)
                v65 = v_pool.tile([128, GW, NH, HD + 1], f32)
                nc.vector.memset(v65[:, :, :, HD : HD + 1], 1.0)
                for w in range(GW):
                    for nk in range(2):
                        ps = psA.tile([128, 512], f32, tag="psA")
                        for k in range(KC):
                            nc.tensor.matmul(
                                ps[:],
                                xT[:].rearrange("p k (g t) -> p k g t", g=GW)[
                                    :, k, w, :
                                ],
                                wq_sb[
                                    :, k, 2 * C + 512 * nk : 2 * C + 512 * (nk + 1)
                                ],
                                start=(k == 0),
                                stop=(k == KC - 1 and not has_qkvb),
                            )
                        if has_qkvb:
                            nc.tensor.matmul(
                                ps[:],
                                ones[0:1, 0:128],
                                qkvb_sb[
                                    0:1, 2 * C + 512 * nk : 2 * C + 512 * (nk + 1)
                                ],
                                start=False,
                                stop=True,
                            )
                        for h8 in range(8):
                            h = 8 * nk + h8
                            nc.scalar.copy(
                                v65[:, w, h, 0:HD], ps[:, 64 * h8 : 64 * (h8 + 1)]
                            )

                # 5+6) attention per (4-head bank, window), then out projection
                for w, (ih, iw) in enumerate(wins):
                    owT = ow_pool.tile([128, KC, 128], fpr)
                    for hb in range(4):
                        psS = psB.tile([128, 512], f32, tag="psB")
                        for m in range(4):
                            h = 4 * hb + m
                            # S^T[kt,qt] = (K_h^T).T @ Q_h^T, K=64, base 0
                            nc.tensor.matmul(
                                psS[:, 128 * m : 128 * (m + 1)],
                                qkT[:, NH + h, 128 * w : 128 * (w + 1)],
                                qkT[:, h, 128 * w : 128 * (w + 1)],
                                start=True,
                                stop=True,
                            )
                        E = e_pool.tile([128, 512], f32, tag="E")
                        nc.scalar.activation(
                            E[:], psS[:], mybir.ActivationFunctionType.Exp
                        )
                        psV = psB.tile([128, 512], f32, tag="psB")
                        for m in range(4):
                            h = 4 * hb + m
                            # rows 0..63 = V^T E (unnormalized), row 64 = denom
                            nc.tensor.matmul(
                                psV[0:65, 128 * m : 128 * (m + 1)],
                                v65[:, w, h, :],
                                E[:, 128 * m : 128 * (m + 1)],
                                start=True,
                                stop=True,
                            )
                        r = r_pool.tile([1, 512], fpr, tag="r")
                        with nc.allow_low_precision(
                            reason="softmax recip rounded to f32r for the "
                            "K=1 broadcast matmul"
                        ):
                            nc.vector.reciprocal(r[:], psV[64:65, :])
                        # broadcast r along partitions: K=1 matmul ones64^T @ r
                        Rp = psB.tile([64, 512], f32, tag="psB")
                        nc.tensor.matmul(Rp[:], ones64[:], r[:], start=True, stop=True)
                        R = r_pool.tile([64, 512], f32, tag="R")
                        nc.scalar.copy(R[:], Rp[:])
                        for m in range(4):
                            h = 4 * hb + m
                            po = (h % 2) * 64
                            nc.vector.tensor_tensor(
                                owT[po : po + 64, h // 2, :],
                                psV[0:64, 128 * m : 128 * (m + 1)],
                                R[:, 128 * m : 128 * (m + 1)],
                                op=mybir.AluOpType.mult,
                            )

                    otile = o_pool.tile([128, C], f32)
                    for nk in range(2):
                        ps = psA.tile([128, 512], f32, tag="psA")
                        for k in range(KC):
                            nc.tensor.matmul(
                                ps[:],
                                owT[:, k, :],
                                wp_sb[:, k, 512 * nk : 512 * (nk + 1)],
                                start=(k == 0),
                                stop=(k == KC - 1 and not has_projb),
                            )
                        if has_projb:
                            nc.tensor.matmul(
                                ps[:],
                                ones[0:1, 0:128],
                                projb_sb[0:1, 512 * nk : 512 * (nk + 1)],
                                start=False,
                                stop=True,
                            )
                        nc.scalar.copy(otile[:, 512 * nk : 512 * (nk + 1)], ps[:])
                    for tt in range(WT):
                        nc.sync.dma_start(
                            out_v[ih, iw, tt], otile[64 * tt : 64 * (tt + 1), :]
                        )

    _split_drain_waits(nc, mybir)
    return nc


def _get_nc(has_qkvb, has_projb):
    key = (has_qkvb, has_projb)
    if key not in _BUILD_CACHE:
        _BUILD_CACHE[key] = _build(has_qkvb, has_projb)
    return _BUILD_CACHE[key]


def kernel(x, qkv_w, qkv_b, proj_w, proj_b, t, h, w, **_unused):
    from concourse.bass_utils import run_bass_kernel_spmd

    x = np.asarray(x, dtype=np.float32)
    qkv_w = np.asarray(qkv_w, dtype=np.float32)
    qkv_b = np.asarray(qkv_b, dtype=np.float32)
    proj_w = np.asarray(proj_w, dtype=np.float32)
    proj_b = np.asarray(proj_b, dtype=np.float32)
    assert x.shape == (B, N, C), x.shape
    assert int(t) == T and int(h) == H and int(w) == W

    has_qkvb = bool(np.any(qkv_b))
    has_projb = bool(np.any(proj_b))
    nc = _get_nc(has_qkvb, has_projb)

    wqkvT = np.ascontiguousarray(qkv_w.T)
    projT = np.ascontiguousarray(proj_w.T)

    in_maps = []
    for core in range(NCORES):
        b, it = divmod(core, T // WT)
        im = {
            "xs": np.ascontiguousarray(x[b, it * SLAB : (it + 1) * SLAB, :]),
            "wqkvT": wqkvT,
            "projT": projT,
        }
        if has_qkvb:
            im["qkvb"] = qkv_b.reshape(1, 3 * C)
        if has_projb:
            im["projb"] = proj_b.reshape(1, C)
        in_maps.append(im)

    res = run_bass_kernel_spmd(nc, in_maps, core_ids=list(range(NCORES)))

    y = np.empty((B, N, C), dtype=np.float32)
    for core in range(NCORES):
        b, it = divmod(core, T // WT)
        y[b, it * SLAB : (it + 1) * SLAB, :] = res.results[core]["out"]
    return y


# revision 19
# speedup vs baseline: 2.0466x; 1.1640x over previous
"""Trainium2 Bass kernel for windowed (block-diagonal) multi-head video attention.

Problem: x:[2,8192,1024] -> qkv proj -> 3D-window (2,8,8) attention over a
(8,32,32) token grid, 16 heads x 64 dim -> out proj -> [2,8192,1024].

Sharding: 8 cores, data-parallel over (batch, t-window-group).  Token order is
(t,h,w)-major, so the slab x[b, it*2048:(it+1)*2048, :] is contiguous and holds
exactly the 16 independent (h,w)-windows with t in {2it, 2it+1}.  Each core:
  - DMA-gathers each window's 128 tokens as a [128,1024] tile (strided AP)
  - PE-transposes x_win -> x^T (contraction dim on partitions)
  - QKV projection: Q,K produced head-transposed [oc,tok]; V token-major with a
    per-head ones column appended (65-stride layout)
  - S^T = K_h Q_h^T per head (K=64), exp on ACT, A·V matmul where the ones row
    yields the softmax denominator for free; normalize with reciprocal +
    gpsimd partition-broadcast + DVE multiply
  - out projection, DMA-scatter back to token order
Weights are pre-transposed on the host; biases (zero in this problem) are
supported via rank-1 (K=1) accumulation matmuls, compiled only when nonzero.
"""

import sys

for _p in ("/opt/trn_rl_repo",):
    if _p not in sys.path:
        sys.path.insert(0, _p)

import numpy as np

B, T, H, W = 2, 8, 32, 32
C, NH, HD = 1024, 16, 64
WT, WH, WW = 2, 8, 8
N = T * H * W              # 8192 tokens
SCALE = HD ** -0.5
NCORES = 8
SLAB = N // (T // WT)      # 2048 tokens per (b, it) slab
NWIN = (H // WH) * (W // WW)   # 16 windows per slab
M = WT * WH * WW           # 128 tokens per window
KC = C // 128              # 8 contraction chunks

_BUILD_CACHE = {}


def _split_drain_waits(nc, mybir, cap=1, event_cap=2):
    """This walrus build accepts only one sem wait per TPB instruction
    (Tile's scheduler attaches up to 3).  Move the excess onto
    InstEventSemaphore carriers (which hold 2) inserted right before the
    over-subscribed instruction on the same engine — the engine blocks on the
    carriers first, so semantics are unchanged."""
    for f in nc.m.functions:
        for bb in f.blocks:
            i = 0
            while i < len(bb.instructions):
                ins = bb.instructions[i]
                si = ins.sync_info
                my_cap = (
                    event_cap
                    if type(ins).__name__ == "InstEventSemaphore"
                    else cap
                )
                if si is not None and si.on_wait and len(si.on_wait) > my_cap:
                    waits = list(si.on_wait)
                    si.on_wait = waits[:my_cap]
                    extra = waits[my_cap:]
                    carriers = []
                    while extra:
                        chunk, extra = extra[:event_cap], extra[event_cap:]
                        ev = mybir.InstEventSemaphore(
                            name=f"I-{nc.next_id()}-waitsplit", ins=[], outs=[]
                        )
                        ev.engine = ins.engine
                        ev.sync_info = mybir.SyncInfo(
                            on_wait=list(chunk), on_update=[]
                        )
                        nc.register_instruction(ev)
                        carriers.append(ev)
                    bb.instructions[i:i] = carriers
                    i += len(carriers)
                i += 1


def _build(has_qkvb, has_projb, use_f32r=True):
    import concourse.bass as bass
    import concourse.tile as tile
    from concourse import mybir
    from concourse.masks import make_identity

    f32 = mybir.dt.float32
    fpr = mybir.dt.float32r if use_f32r else f32

    nc = bass.Bass("TRN2", target_bir_lowering=False, debug=False)
    xs = nc.dram_tensor("xs", [SLAB, C], f32, kind="ExternalInput")
    # weight dtype f32r: same 4-byte layout, PE rounds on read (tf32-like)
    wqkvT = nc.dram_tensor("wqkvT", [C, 3 * C], fpr, kind="ExternalInput")
    projT = nc.dram_tensor("projT", [C, C], fpr, kind="ExternalInput")
    if has_qkvb:
        qkvb = nc.dram_tensor("qkvb", [1, 3 * C], fpr, kind="ExternalInput")
    if has_projb:
        projb = nc.dram_tensor("projb", [1, C], fpr, kind="ExternalInput")
    out = nc.dram_tensor("out", [SLAB, C], f32, kind="ExternalOutput")

    # window gather/scatter views: slab token idx = tt*1024 + hh*32 + ww in a
    # [2, (4,8), (4,8)] = (tt, ih hh, iw ww) decomposition; window = (ih, iw)
    xs_v = xs.ap().rearrange(
        "(tt ih hh iw ww) c -> ih iw tt hh ww c", tt=WT, ih=4, hh=WH, iw=4, ww=WW
    )
    out_v = out.ap().rearrange(
        "(tt ih hh iw ww) c -> ih iw tt hh ww c", tt=WT, ih=4, hh=WH, iw=4, ww=WW
    )

    # windows processed in pairs: tok dim = 256 so the f32r matmuls hit the
    # 1 cyc/row regime (ap_size >= 256); attention blocks stay per-window
    GW = 2
    TOKG = 128 * GW

    with tile.TileContext(nc) as tc:
        with (
            tc.tile_pool(name="wq", bufs=1) as wq_pool,
            tc.tile_pool(name="wp", bufs=1) as wp_pool,
            tc.tile_pool(name="const", bufs=1) as const_pool,
            tc.tile_pool(name="xw", bufs=1) as xw_pool,
            tc.tile_pool(name="xT", bufs=1) as xT_pool,
            tc.tile_pool(name="qk", bufs=1) as qk_pool,
            tc.tile_pool(name="v65", bufs=1) as v_pool,
            tc.tile_pool(name="E", bufs=2) as e_pool,
            tc.tile_pool(name="rR", bufs=2) as r_pool,
            tc.tile_pool(name="owT", bufs=1) as ow_pool,
            tc.tile_pool(name="o", bufs=1) as o_pool,
            tc.tile_pool(name="psA", bufs=4, space="PSUM") as psA,
            tc.tile_pool(name="psB", bufs=4, space="PSUM") as psB,
        ):
            ident = const_pool.tile([128, 128], f32)
            make_identity(nc, ident[:])
            ones_col = const_pool.tile([128, GW * NH], f32)
            nc.vector.memset(ones_col[:], 1.0)
            ones64f = const_pool.tile([1, 64], f32)
            nc.vector.memset(ones64f[:], 1.0)
            ones64 = const_pool.tile([1, 64], fpr)
            nc.scalar.copy(ones64[:], ones64f[:])

            wq_sb = wq_pool.tile([128, KC, 3 * C], fpr)
            nc.sync.dma_start(
                wq_sb[:], wqkvT.ap().rearrange("(k p) o -> p k o", p=128)
            )
            wp_sb = wp_pool.tile([128, KC, C], fpr)
            nc.sync.dma_start(
                wp_sb[:], projT.ap().rearrange("(k p) o -> p k o", p=128)
            )
            if has_qkvb or has_projb:
                onesf = const_pool.tile([1, 128], f32)
                nc.vector.memset(onesf[:], 1.0)
                ones = const_pool.tile([1, 128], fpr)
                nc.scalar.copy(ones[:], onesf[:])
            if has_qkvb:
                qkvb_sb = const_pool.tile([1, 3 * C], fpr)
                nc.sync.dma_start(qkvb_sb[:], qkvb.ap())
            if has_projb:
                projb_sb = const_pool.tile([1, C], fpr)
                nc.sync.dma_start(projb_sb[:], projb.ap())

            for grp in range(NWIN // GW):
                wins = [(divmod(GW * grp + w, 4)) for w in range(GW)]

                # 1+2) per window: gather tokens, PE-transpose into the
                # group x^T tile [c-chunk partitions, (chunk, tok)] (f32r)
                xT = xT_pool.tile([128, KC, TOKG], fpr)
                for w, (ih, iw) in enumerate(wins):
                    xw = xw_pool.tile([128, C], f32)
                    for tt in range(WT):
                        nc.sync.dma_start(
                            xw[64 * tt : 64 * (tt + 1), :], xs_v[ih, iw, tt]
                        )
                    for tb in range(2):
                        ps = psA.tile([128, 512], f32, tag="psA")
                        for j in range(4):
                            jj = 4 * tb + j
                            nc.tensor.transpose(
                                ps[:, 128 * j : 128 * (j + 1)],
                                xw[:, 128 * jj : 128 * (jj + 1)],
                                ident[:],
                            )
                        psv = ps[:].rearrange("p (c t) -> p c t", t=128)
                        nc.scalar.copy(
                            xT[:].rearrange("p k (g t) -> p k g t", g=GW)[
                                :, 4 * tb : 4 * tb + 4, w, :
                            ],
                            psv[:],
                        )

                # 3) Q,K head-transposed: psum bank [oc 128, tok 256] x2 chunks.
                # Evict to 64-partition per-head layout (slot 2c+parity) so S
                # matmuls never use partition-base-64 operands (mixing base-0
                # and base-64 matmul operands hangs trn2).  qkT is f32r so the
                # S matmuls run as a single (rounded) pass instead of fp32's
                # HI+LO pair.
                qkT = qk_pool.tile([64, 4 * KC, TOKG], fpr)
                qkTv = qkT[:].rearrange("p (s two) t -> p s two t", two=2)
                for bank in range(8):
                    ps = psA.tile([128, 512], f32, tag="psA")
                    for sub in range(2):
                        oc = 2 * bank + sub
                        for k in range(KC):
                            nc.tensor.matmul(
                                ps[:, TOKG * sub : TOKG * (sub + 1)],
                                wq_sb[:, k, 128 * oc : 128 * (oc + 1)],
                                xT[:, k, :],
                                start=(k == 0),
                                stop=(k == KC - 1 and not has_qkvb),
                            )
                        if has_qkvb:
                            nc.tensor.matmul(
                                ps[:, TOKG * sub : TOKG * (sub + 1)],
                                qkvb_sb[0:1, 128 * oc : 128 * (oc + 1)],
                                ones[0:1, 0:TOKG],
                                start=False,
                                stop=True,
                            )
                    sc = SCALE if bank < 4 else 1.0
                    psv = ps[:].rearrange("p (c t) -> p c t", t=TOKG)
                    with nc.allow_low_precision(reason="f32r eviction"):
                        nc.vector.tensor_scalar_mul(
                            qkTv[:, 2 * bank : 2 * bank + 2, 0, :],
                            psv[0:64, :, :],
                            sc,
                        )
                        nc.vector.tensor_scalar_mul(
                            qkTv[:, 2 * bank : 2 * bank + 2, 1, :],
                            psv[64:128, :, :],
                            sc,
                        )

                # 4) V token-major per window, ones column per head (stride 65)
                v65 = v_pool.tile([128, GW, NH, HD + 1], fpr)
                nc.scalar.copy(
                    v65[:, :, :, HD : HD + 1],
                    ones_col[:].rearrange("p (g h) -> p g h", g=GW)[:, :, :, None],
                )
                for w in range(GW):
                    for nk in range(2):
                        ps = psA.tile([128, 512], f32, tag="psA")
                        for k in range(KC):
                            nc.tensor.matmul(
                                ps[:],
                                xT[:].rearrange("p k (g t) -> p k g t", g=GW)[
                                    :, k, w, :
                                ],
                                wq_sb[
                                    :, k, 2 * C + 512 * nk : 2 * C + 512 * (nk + 1)
                                ],
                                start=(k == 0),
                                stop=(k == KC - 1 and not has_qkvb),
                            )
                        if has_qkvb:
                            nc.tensor.matmul(
                                ps[:],
                                ones[0:1, 0:128],
                                qkvb_sb[
                                    0:1, 2 * C + 512 * nk : 2 * C + 512 * (nk + 1)
                                ],
                                start=False,
                                stop=True,
                            )
                        # one strided eviction for all 8 heads of this bank
                        nc.scalar.copy(
                            v65[:, w, 8 * nk : 8 * nk + 8, 0:HD],
                            ps[:].rearrange("p (h e) -> p h e", e=HD),
                        )

                # 5+6) attention per (4-head bank, window), then out projection
                for w, (ih, iw) in enumerate(wins):
                    owT = ow_pool.tile([128, KC, 128], fpr)
                    # all 4 S banks first so exp/AV overlap the S matmuls
                    psS_banks = []
                    for hb in range(4):
                        psS = psB.tile([128, 512], f32, tag="psB")
                        for m in range(4):
                            h = 4 * hb + m
                            # S^T[kt,qt] = (K_h^T).T @ Q_h^T, K=64, base 0
                            nc.tensor.matmul(
                                psS[:, 128 * m : 128 * (m + 1)],
                                qkT[:, NH + h, 128 * w : 128 * (w + 1)],
                                qkT[:, h, 128 * w : 128 * (w + 1)],
                                start=True,
                                stop=True,
                            )
                        psS_banks.append(psS)
                    for hb in range(4):
                        E = e_pool.tile([128, 512], fpr, tag="E")
                        with nc.allow_low_precision(reason="f32r attn weights"):
                            nc.scalar.activation(
                                E[:],
                                psS_banks[hb][:],
                                mybir.ActivationFunctionType.Exp,
                            )
                        psV = psA.tile([128, 512], f32, tag="psA")
                        for m in range(4):
                            h = 4 * hb + m
                            # rows 0..63 = V^T E (unnormalized), row 64 = denom
                            nc.tensor.matmul(
                                psV[0:65, 128 * m : 128 * (m + 1)],
                                v65[:, w, h, :],
                                E[:, 128 * m : 128 * (m + 1)],
                                start=True,
                                stop=True,
                            )
                        # denom -> SBUF, broadcast via K=1 matmul, THEN
                        # reciprocal on [64,512] (64 lanes; a [1,512] recip
                        # serializes 512 elements on one DVE lane at ~3.3us)
                        den = r_pool.tile([1, 512], fpr, tag="r")
                        with nc.allow_low_precision(reason="f32r denom"):
                            nc.scalar.copy(den[:], psV[64:65, :])
                        Rp = psA.tile([64, 512], f32, tag="psA")
                        nc.tensor.matmul(
                            Rp[:], ones64[:], den[:], start=True, stop=True
                        )
                        R = r_pool.tile([64, 512], f32, tag="R")
                        nc.vector.reciprocal(R[:], Rp[:])
                        for m in range(4):
                            h = 4 * hb + m
                            po = (h % 2) * 64
                            nc.vector.tensor_tensor(
                                owT[po : po + 64, h // 2, :],
                                psV[0:64, 128 * m : 128 * (m + 1)],
                                R[:, 128 * m : 128 * (m + 1)],
                                op=mybir.AluOpType.mult,
                            )

                    otile = o_pool.tile([128, C], f32)
                    for nk in range(2):
                        ps = psA.tile([128, 512], f32, tag="psA")
                        for k in range(KC):
                            nc.tensor.matmul(
                                ps[:],
                                owT[:, k, :],
                                wp_sb[:, k, 512 * nk : 512 * (nk + 1)],
                                start=(k == 0),
                                stop=(k == KC - 1 and not has_projb),
                            )
                        if has_projb:
                            nc.tensor.matmul(
                                ps[:],
                                ones[0:1, 0:128],
                                projb_sb[0:1, 512 * nk : 512 * (nk + 1)],
                                start=False,
                                stop=True,
                            )
                        nc.scalar.copy(otile[:, 512 * nk : 512 * (nk + 1)], ps[:])
                    for tt in range(WT):
                        nc.sync.dma_start(
                            out_v[ih, iw, tt], otile[64 * tt : 64 * (tt + 1), :]
                        )

    _split_drain_waits(nc, mybir)
    return nc


def _get_nc(has_qkvb, has_projb):
    key = (has_qkvb, has_projb)
    if key not in _BUILD_CACHE:
        _BUILD_CACHE[key] = _build(has_qkvb, has_projb)
    return _BUILD_CACHE[key]


def kernel(x, qkv_w, qkv_b, proj_w, proj_b, t, h, w, **_unused):
    from concourse.bass_utils import run_bass_kernel_spmd

    x = np.asarray(x, dtype=np.float32)
    qkv_w = np.asarray(qkv_w, dtype=np.float32)
    qkv_b = np.asarray(qkv_b, dtype=np.float32)
    proj_w = np.asarray(proj_w, dtype=np.float32)
    proj_b = np.asarray(proj_b, dtype=np.float32)
    assert x.shape == (B, N, C), x.shape
    assert int(t) == T and int(h) == H and int(w) == W

    has_qkvb = bool(np.any(qkv_b))
    has_projb = bool(np.any(proj_b))
    nc = _get_nc(has_qkvb, has_projb)

    wqkvT = np.ascontiguousarray(qkv_w.T)
    projT = np.ascontiguousarray(proj_w.T)

    in_maps = []
    for core in range(NCORES):
        b, it = divmod(core, T // WT)
        im = {
            "xs": np.ascontiguousarray(x[b, it * SLAB : (it + 1) * SLAB, :]),
            "wqkvT": wqkvT,
            "projT": projT,
        }
        if has_qkvb:
            im["qkvb"] = qkv_b.reshape(1, 3 * C)
        if has_projb:
            im["projb"] = proj_b.reshape(1, C)
        in_maps.append(im)

    res = run_bass_kernel_spmd(nc, in_maps, core_ids=list(range(NCORES)))

    y = np.empty((B, N, C), dtype=np.float32)
    for core in range(NCORES):
        b, it = divmod(core, T // WT)
        y[b, it * SLAB : (it + 1) * SLAB, :] = res.results[core]["out"]
    return y


# revision 22
# speedup vs baseline: 2.5436x; 1.2429x over previous
"""Trainium2 Bass kernel for windowed (block-diagonal) multi-head video attention.

Problem: x:[2,8192,1024] -> qkv proj -> 3D-window (2,8,8) attention over a
(8,32,32) token grid, 16 heads x 64 dim -> out proj -> [2,8192,1024].

Sharding: 8 cores, data-parallel over (batch, t-window-group).  Token order is
(t,h,w)-major, so the slab x[b, it*2048:(it+1)*2048, :] is contiguous and holds
exactly the 16 independent (h,w)-windows with t in {2it, 2it+1}.  Each core:
  - DMA-gathers each window's 128 tokens as a [128,1024] tile (strided AP)
  - PE-transposes x_win -> x^T (contraction dim on partitions)
  - QKV projection: Q,K produced head-transposed [oc,tok]; V token-major with a
    per-head ones column appended (65-stride layout)
  - S^T = K_h Q_h^T per head (K=64), exp on ACT, A·V matmul where the ones row
    yields the softmax denominator for free; normalize with reciprocal +
    gpsimd partition-broadcast + DVE multiply
  - out projection, DMA-scatter back to token order
Weights are pre-transposed on the host; biases (zero in this problem) are
supported via rank-1 (K=1) accumulation matmuls, compiled only when nonzero.
"""

import sys

for _p in ("/opt/trn_rl_repo",):
    if _p not in sys.path:
        sys.path.insert(0, _p)

import numpy as np

B, T, H, W = 2, 8, 32, 32
C, NH, HD = 1024, 16, 64
WT, WH, WW = 2, 8, 8
N = T * H * W              # 8192 tokens
SCALE = HD ** -0.5
NCORES = 8
SLAB = N // (T // WT)      # 2048 tokens per (b, it) slab
NWIN = (H // WH) * (W // WW)   # 16 windows per slab
M = WT * WH * WW           # 128 tokens per window
KC = C // 128              # 8 contraction chunks

_BUILD_CACHE = {}


def _split_drain_waits(nc, mybir, cap=1, event_cap=2):
    """This walrus build accepts only one sem wait per TPB instruction
    (Tile's scheduler attaches up to 3).  Move the excess onto
    InstEventSemaphore carriers (which hold 2) inserted right before the
    over-subscribed instruction on the same engine — the engine blocks on the
    carriers first, so semantics are unchanged."""
    for f in nc.m.functions:
        for bb in f.blocks:
            i = 0
            while i < len(bb.instructions):
                ins = bb.instructions[i]
                si = ins.sync_info
                my_cap = (
                    event_cap
                    if type(ins).__name__ == "InstEventSemaphore"
                    else cap
                )
                if si is not None and si.on_wait and len(si.on_wait) > my_cap:
                    waits = list(si.on_wait)
                    si.on_wait = waits[:my_cap]
                    extra = waits[my_cap:]
                    carriers = []
                    while extra:
                        chunk, extra = extra[:event_cap], extra[event_cap:]
                        ev = mybir.InstEventSemaphore(
                            name=f"I-{nc.next_id()}-waitsplit", ins=[], outs=[]
                        )
                        ev.engine = ins.engine
                        ev.sync_info = mybir.SyncInfo(
                            on_wait=list(chunk), on_update=[]
                        )
                        nc.register_instruction(ev)
                        carriers.append(ev)
                    bb.instructions[i:i] = carriers
                    i += len(carriers)
                i += 1


def _build(has_qkvb, has_projb, use_f32r=True):
    import concourse.bass as bass
    import concourse.tile as tile
    from concourse import mybir
    from concourse.masks import make_identity

    f32 = mybir.dt.float32
    fpr = mybir.dt.float32r if use_f32r else f32

    nc = bass.Bass("TRN2", target_bir_lowering=False, debug=False)
    xs = nc.dram_tensor("xs", [SLAB, C], f32, kind="ExternalInput")
    # weight dtype f32r: same 4-byte layout, PE rounds on read (tf32-like)
    wqkvT = nc.dram_tensor("wqkvT", [C, 3 * C], fpr, kind="ExternalInput")
    projT = nc.dram_tensor("projT", [C, C], fpr, kind="ExternalInput")
    if has_qkvb:
        qkvb = nc.dram_tensor("qkvb", [1, 3 * C], fpr, kind="ExternalInput")
    if has_projb:
        projb = nc.dram_tensor("projb", [1, C], fpr, kind="ExternalInput")
    out = nc.dram_tensor("out", [SLAB, C], f32, kind="ExternalOutput")

    # window gather/scatter views: slab token idx = tt*1024 + hh*32 + ww in a
    # [2, (4,8), (4,8)] = (tt, ih hh, iw ww) decomposition; window = (ih, iw)
    xs_v = xs.ap().rearrange(
        "(tt ih hh iw ww) c -> ih iw tt hh ww c", tt=WT, ih=4, hh=WH, iw=4, ww=WW
    )
    out_v = out.ap().rearrange(
        "(tt ih hh iw ww) c -> ih iw tt hh ww c", tt=WT, ih=4, hh=WH, iw=4, ww=WW
    )

    # windows processed in pairs: tok dim = 256 so the f32r matmuls hit the
    # 1 cyc/row regime (ap_size >= 256); attention blocks stay per-window
    GW = 2
    TOKG = 128 * GW

    with tile.TileContext(nc) as tc:
        with (
            tc.tile_pool(name="wq", bufs=1) as wq_pool,
            tc.tile_pool(name="wp", bufs=1) as wp_pool,
            tc.tile_pool(name="const", bufs=1) as const_pool,
            tc.tile_pool(name="xw", bufs=2) as xw_pool,
            tc.tile_pool(name="xT", bufs=1) as xT_pool,
            tc.tile_pool(name="qk", bufs=1) as qk_pool,
            tc.tile_pool(name="v65", bufs=1) as v_pool,
            tc.tile_pool(name="E", bufs=2) as e_pool,
            tc.tile_pool(name="rR", bufs=2) as r_pool,
            tc.tile_pool(name="owT", bufs=1) as ow_pool,
            tc.tile_pool(name="o", bufs=1) as o_pool,
            tc.tile_pool(name="psA", bufs=4, space="PSUM") as psA,
            tc.tile_pool(name="psB", bufs=4, space="PSUM") as psB,
        ):
            ident = const_pool.tile([128, 128], f32)
            make_identity(nc, ident[:])
            ones_col = const_pool.tile([128, GW * NH], f32)
            nc.vector.memset(ones_col[:], 1.0)
            ones64f = const_pool.tile([1, 64], f32)
            nc.vector.memset(ones64f[:], 1.0)
            ones64 = const_pool.tile([1, 64], fpr)
            nc.scalar.copy(ones64[:], ones64f[:])

            wq_sb = wq_pool.tile([128, KC, 3 * C], fpr)
            nc.sync.dma_start(
                wq_sb[:], wqkvT.ap().rearrange("(k p) o -> p k o", p=128)
            )
            wp_sb = wp_pool.tile([128, KC, C], fpr)
            nc.sync.dma_start(
                wp_sb[:], projT.ap().rearrange("(k p) o -> p k o", p=128)
            )
            if has_qkvb or has_projb:
                onesf = const_pool.tile([1, 128], f32)
                nc.vector.memset(onesf[:], 1.0)
                ones = const_pool.tile([1, 128], fpr)
                nc.scalar.copy(ones[:], onesf[:])
            if has_qkvb:
                qkvb_sb = const_pool.tile([1, 3 * C], fpr)
                nc.sync.dma_start(qkvb_sb[:], qkvb.ap())
            if has_projb:
                projb_sb = const_pool.tile([1, C], fpr)
                nc.sync.dma_start(projb_sb[:], projb.ap())

            for grp in range(NWIN // GW):
                wins = [(divmod(GW * grp + w, 4)) for w in range(GW)]

                # 1+2) per window: gather tokens, PE-transpose into the
                # group x^T tile [c-chunk partitions, (chunk, tok)] (f32r)
                xT = xT_pool.tile([128, KC, TOKG], fpr)
                for w, (ih, iw) in enumerate(wins):
                    xw = xw_pool.tile([128, C], f32)
                    for tt in range(WT):
                        nc.scalar.dma_start(
                            xw[64 * tt : 64 * (tt + 1), :], xs_v[ih, iw, tt]
                        )
                    for tb in range(2):
                        ps = psA.tile([128, 512], f32, tag="psA")
                        for j in range(4):
                            jj = 4 * tb + j
                            nc.tensor.transpose(
                                ps[:, 128 * j : 128 * (j + 1)],
                                xw[:, 128 * jj : 128 * (jj + 1)],
                                ident[:],
                            )
                        psv = ps[:].rearrange("p (c t) -> p c t", t=128)
                        nc.scalar.copy(
                            xT[:].rearrange("p k (g t) -> p k g t", g=GW)[
                                :, 4 * tb : 4 * tb + 4, w, :
                            ],
                            psv[:],
                        )

                # 3) Q,K head-transposed: psum bank [oc 128, tok 256] x2 chunks.
                # Evict to 64-partition per-head layout (slot 2c+parity) so S
                # matmuls never use partition-base-64 operands (mixing base-0
                # and base-64 matmul operands hangs trn2).  qkT is f32r so the
                # S matmuls run as a single (rounded) pass instead of fp32's
                # HI+LO pair.
                qkT = qk_pool.tile([64, 4 * KC, TOKG], fpr)
                qkTv = qkT[:].rearrange("p (s two) t -> p s two t", two=2)
                for bank in range(8):
                    ps = psA.tile([128, 512], f32, tag="psA")
                    for sub in range(2):
                        oc = 2 * bank + sub
                        for k in range(KC):
                            nc.tensor.matmul(
                                ps[:, TOKG * sub : TOKG * (sub + 1)],
                                wq_sb[:, k, 128 * oc : 128 * (oc + 1)],
                                xT[:, k, :],
                                start=(k == 0),
                                stop=(k == KC - 1 and not has_qkvb),
                            )
                        if has_qkvb:
                            nc.tensor.matmul(
                                ps[:, TOKG * sub : TOKG * (sub + 1)],
                                qkvb_sb[0:1, 128 * oc : 128 * (oc + 1)],
                                ones[0:1, 0:TOKG],
                                start=False,
                                stop=True,
                            )
                    sc = SCALE if bank < 4 else 1.0
                    psv = ps[:].rearrange("p (c t) -> p c t", t=TOKG)
                    with nc.allow_low_precision(reason="f32r eviction"):
                        nc.vector.tensor_scalar_mul(
                            qkTv[:, 2 * bank : 2 * bank + 2, 0, :],
                            psv[0:64, :, :],
                            sc,
                        )
                        nc.vector.tensor_scalar_mul(
                            qkTv[:, 2 * bank : 2 * bank + 2, 1, :],
                            psv[64:128, :, :],
                            sc,
                        )

                # 4) V token-major per window, ones column per head (stride 65)
                v65 = v_pool.tile([128, GW, NH, HD + 1], fpr)
                nc.scalar.copy(
                    v65[:, :, :, HD : HD + 1],
                    ones_col[:].rearrange("p (g h) -> p g h", g=GW)[:, :, :, None],
                )
                for w in range(GW):
                    for nk in range(2):
                        ps = psA.tile([128, 512], f32, tag="psA")
                        for half in range(2):
                            lo = 2 * C + 512 * nk + 256 * half
                            for k in range(KC):
                                nc.tensor.matmul(
                                    ps[:, 256 * half : 256 * (half + 1)],
                                    xT[:].rearrange(
                                        "p k (g t) -> p k g t", g=GW
                                    )[:, k, w, :],
                                    wq_sb[:, k, lo : lo + 256],
                                    start=(k == 0),
                                    stop=(k == KC - 1 and not has_qkvb),
                                )
                            if has_qkvb:
                                nc.tensor.matmul(
                                    ps[:, 256 * half : 256 * (half + 1)],
                                    ones[0:1, 0:128],
                                    qkvb_sb[0:1, lo : lo + 256],
                                    start=False,
                                    stop=True,
                                )
                        # one strided eviction for all 8 heads of this bank
                        nc.scalar.copy(
                            v65[:, w, 8 * nk : 8 * nk + 8, 0:HD],
                            ps[:].rearrange("p (h e) -> p h e", e=HD),
                        )

                # 5+6) attention per (4-head bank, window), then out projection
                for w, (ih, iw) in enumerate(wins):
                    owT = ow_pool.tile([128, KC, 128], fpr)
                    # all 4 S banks first so exp/AV overlap the S matmuls
                    psS_banks = []
                    for hb in range(4):
                        psS = psB.tile([128, 512], f32, tag="psB")
                        for m in range(4):
                            h = 4 * hb + m
                            # S^T[kt,qt] = (K_h^T).T @ Q_h^T, K=64, base 0
                            nc.tensor.matmul(
                                psS[:, 128 * m : 128 * (m + 1)],
                                qkT[:, NH + h, 128 * w : 128 * (w + 1)],
                                qkT[:, h, 128 * w : 128 * (w + 1)],
                                start=True,
                                stop=True,
                            )
                        psS_banks.append(psS)
                    for hb in range(4):
                        E = e_pool.tile([128, 512], fpr, tag="E")
                        with nc.allow_low_precision(reason="f32r attn weights"):
                            nc.scalar.activation(
                                E[:],
                                psS_banks[hb][:],
                                mybir.ActivationFunctionType.Exp,
                            )
                        psV = psA.tile([128, 512], f32, tag="psA")
                        for m in range(4):
                            h = 4 * hb + m
                            # rows 0..63 = V^T E (unnormalized), row 64 = denom
                            nc.tensor.matmul(
                                psV[0:65, 128 * m : 128 * (m + 1)],
                                v65[:, w, h, :],
                                E[:, 128 * m : 128 * (m + 1)],
                                start=True,
                                stop=True,
                            )
                        # softmax 1/denom as exp(-ln(den)) on the ACT
                        # tables (InstReciprocal costs ~9 cyc/elem/lane and
                        # the denom row is a single-partition [1,512]);
                        # then partition-broadcast via a K=1 matmul
                        L = r_pool.tile([1, 512], f32, tag="r")
                        nc.scalar.activation(
                            L[:], psV[64:65, :], mybir.ActivationFunctionType.Ln
                        )
                        r = r_pool.tile([1, 512], fpr, tag="r")
                        with nc.allow_low_precision(reason="f32r recip"):
                            nc.scalar.activation(
                                r[:],
                                L[:],
                                mybir.ActivationFunctionType.Exp,
                                scale=-1.0,
                            )
                        Rp = psA.tile([64, 512], f32, tag="psA")
                        for half in range(2):
                            nc.tensor.matmul(
                                Rp[:, 256 * half : 256 * (half + 1)],
                                ones64[:],
                                r[0:1, 256 * half : 256 * (half + 1)],
                                start=True,
                                stop=True,
                            )
                        R = r_pool.tile([64, 512], f32, tag="R")
                        nc.scalar.copy(R[:], Rp[:])
                        for m in range(4):
                            h = 4 * hb + m
                            po = (h % 2) * 64
                            nc.vector.tensor_tensor(
                                owT[po : po + 64, h // 2, :],
                                psV[0:64, 128 * m : 128 * (m + 1)],
                                R[:, 128 * m : 128 * (m + 1)],
                                op=mybir.AluOpType.mult,
                            )

                    otile = o_pool.tile([128, C], f32)
                    for nk in range(2):
                        ps = psA.tile([128, 512], f32, tag="psA")
                        for half in range(2):
                            lo = 512 * nk + 256 * half
                            for k in range(KC):
                                nc.tensor.matmul(
                                    ps[:, 256 * half : 256 * (half + 1)],
                                    owT[:, k, :],
                                    wp_sb[:, k, lo : lo + 256],
                                    start=(k == 0),
                                    stop=(k == KC - 1 and not has_projb),
                                )
                            if has_projb:
                                nc.tensor.matmul(
                                    ps[:, 256 * half : 256 * (half + 1)],
                                    ones[0:1, 0:128],
                                    projb_sb[0:1, lo : lo + 256],
                                    start=False,
                                    stop=True,
                                )
                        nc.scalar.copy(otile[:, 512 * nk : 512 * (nk + 1)], ps[:])
                    for tt in range(WT):
                        nc.sync.dma_start(
                            out_v[ih, iw, tt], otile[64 * tt : 64 * (tt + 1), :]
                        )

    _split_drain_waits(nc, mybir)
    return nc


def _get_nc(has_qkvb, has_projb):
    key = (has_qkvb, has_projb)
    if key not in _BUILD_CACHE:
        _BUILD_CACHE[key] = _build(has_qkvb, has_projb)
    return _BUILD_CACHE[key]


def kernel(x, qkv_w, qkv_b, proj_w, proj_b, t, h, w, **_unused):
    from concourse.bass_utils import run_bass_kernel_spmd

    x = np.asarray(x, dtype=np.float32)
    qkv_w = np.asarray(qkv_w, dtype=np.float32)
    qkv_b = np.asarray(qkv_b, dtype=np.float32)
    proj_w = np.asarray(proj_w, dtype=np.float32)
    proj_b = np.asarray(proj_b, dtype=np.float32)
    assert x.shape == (B, N, C), x.shape
    assert int(t) == T and int(h) == H and int(w) == W

    has_qkvb = bool(np.any(qkv_b))
    has_projb = bool(np.any(proj_b))
    nc = _get_nc(has_qkvb, has_projb)

    wqkvT = np.ascontiguousarray(qkv_w.T)
    projT = np.ascontiguousarray(proj_w.T)

    in_maps = []
    for core in range(NCORES):
        b, it = divmod(core, T // WT)
        im = {
            "xs": np.ascontiguousarray(x[b, it * SLAB : (it + 1) * SLAB, :]),
            "wqkvT": wqkvT,
            "projT": projT,
        }
        if has_qkvb:
            im["qkvb"] = qkv_b.reshape(1, 3 * C)
        if has_projb:
            im["projb"] = proj_b.reshape(1, C)
        in_maps.append(im)

    res = run_bass_kernel_spmd(nc, in_maps, core_ids=list(range(NCORES)))

    y = np.empty((B, N, C), dtype=np.float32)
    for core in range(NCORES):
        b, it = divmod(core, T // WT)
        y[b, it * SLAB : (it + 1) * SLAB, :] = res.results[core]["out"]
    return y
